# revision 20
# baseline (speedup 1.0000x reference)
"""Trainium2 Bass kernel for nn_Dense_test_1layer (DH-SNN dense 1-layer).

Self-contained: hardcodes shapes/sharding. 8-core pure data parallelism over
batch (16 samples/core). All state kept in "T layout": channels on SBUF
partitions, batch on the free dim.

Math decomposition (see host_prep):
  k_t = [x_t, spk_{t-1}],  lin = Wm@k + b1, Wm = W1*mask = [Wx | Ws]
  dinp_t = beta*dinp_{t-1} + (1-beta)*lin_t          (256 dendritic channels)
  Split dinp = dinpA + dinp0:
    dinpA_t = beta*dinpA_{t-1} + Ad_t, Ad_t = (1-beta)*(Wx@x_t + b1)  [precomputed:
       big bf16(hi+lo) matmul over all timesteps + tensor_tensor_scan]
    dinp0_t = beta*dinp0_{t-1} + Ws'@spk_{t-1}, Ws' = (1-beta)*Ws     [loop matmuls]
  mem1_t = alpha*mem1 + (1-alpha)*sum_br(dinp_t) - spk_{t-1}
         = diaga@mem1 + Sb0@dinp0_h0 + Sb1@dinp0_h1 + (S@Ws'*(1-a) - I)@spk + I@SdA_t
    with SdA_t = (1-alpha)*S@dinpA_t precomputed.
  spk_t = (mem1_t > 1)                                [one tensor_scalar per step]
  mem2_t = a2*mem2 + (1-a2)*(W2@spk_t)               [post-loop matmul + scan]
  out = sum_{t>10} softmax(mem2_t)                   [batched exp/colsum-matmul/reduce]
"""

import numpy as np
import ml_dtypes

import orjson

import concourse.bass as bass
import concourse.tile as tile
from concourse import mybir, bass2jax
from concourse.bass_utils import run_bass_kernel_spmd


# --- workaround: this walrus build supports only ONE sync-wait per
# instruction; Tile emits up to ~3. Split excess waits onto injected NoOps
# (same engine queue, in-order, so semantics are preserved).
def _split_waits(bir_json: bytes, max_waits: int = 1) -> bytes:
    d = orjson.loads(bir_json)
    changed = False
    for f in d["functions"]:
        for bb in f.get("blocks", []):
            out = []
            for ins in bb.get("instructions", []):
                si = ins.get("sync_info")
                waits = (si or {}).get("on_wait") or []
                if len(waits) > max_waits and ins.get("opcode") != "ISA":
                    changed = True
                    extra, keep = waits[:-max_waits], waits[-max_waits:]
                    for i in range(0, len(extra), max_waits):
                        out.append({
                            "debug": ins.get("debug", 0),
                            "engine": ins["engine"],
                            "ins": [], "outs": [],
                            "name": f"{ins['name']}-w{i}",
                            "opcode": "NoOp",
                            "sync_info": {"on_update": [],
                                          "on_wait": extra[i:i + max_waits]},
                        })
                    si["on_wait"] = keep
                out.append(ins)
            bb["instructions"] = out
    return orjson.dumps(d) if changed else bir_json


_orig_compile_bir_kernel = bass2jax.compile_bir_kernel


def _patched_compile_bir_kernel(bir_json, tmpdir, neff_name="file.neff"):
    return _orig_compile_bir_kernel(_split_waits(bir_json), tmpdir, neff_name=neff_name)


if bass2jax.compile_bir_kernel is not _patched_compile_bir_kernel:
    bass2jax.compile_bir_kernel = _patched_compile_bir_kernel

F32 = mybir.dt.float32
BF16 = mybir.dt.bfloat16
AL = mybir.AluOpType
AF = mybir.ActivationFunctionType

B, T_FULL, D, N, C, BR = 128, 500, 700, 64, 20, 4
NB = 16            # batch per core
CH = N * BR        # 256 dendritic channels
KAUG = 704         # 700 x-channels + 1 bias row + 3 pad
KCH = [128, 128, 128, 128, 128, 64]   # contraction chunks of KAUG
NCORES = 8
VTH = 1.0


# ----------------------------------------------------------------- host math
def _mkspec(entries):
    out, c0 = [], 0
    for k, p, w in entries:
        out.append((k, p, w, c0)); c0 += w
    return out, c0

_SPEC32, _W32 = _mkspec([
    ("sb0", 128, N),
    ("sb1", 128, N), ("diagb0", 128, 128), ("diagb1", 128, 128),
    ("diaga", N, N), ("selw0", 128, N), ("selw1", 128, N), ("jcc", C, C),
    ("beta0", 128, 1), ("beta1", 128, 1), ("omb0", 128, 1), ("omb1", 128, 1),
    ("alpha2", C, 1), ("alpha1", N, 1)])
_SPEC16, _W16 = _mkspec(
    [(f"wxhi{kc}", KCH[kc], CH) for kc in range(6)]
    + [(f"wxlo{kc}", KCH[kc], CH) for kc in range(6)]
    + [("w2hi", N, C), ("w2lo", N, C), ("wsh0", N, 128), ("wsh1", N, 128),
       ("swsmi", N, N)])



def _sig(v):
    return (1.0 / (1.0 + np.exp(-v.astype(np.float64)))).astype(np.float32)


def host_prep(W1, b1, mask, tau_m1, tau_n1, W2, b2, tau_m2):
    """All weight folding on host. Returns dict of numpy arrays (shared across cores)."""
    alpha = _sig(np.asarray(tau_m1))                    # (64,)
    beta = _sig(np.asarray(tau_n1)).reshape(CH)         # (256,) ch = n*BR+br
    alpha2 = _sig(np.asarray(tau_m2))                   # (20,)
    Wm = (np.asarray(W1) * np.asarray(mask)).astype(np.float32)
    Wx, Ws = Wm[:, :D], Wm[:, D:]
    omb = 1.0 - beta
    oma = 1.0 - alpha
    Wsp = omb[:, None] * Ws                             # (256,64)
    S = np.zeros((N, CH), np.float32)
    for n in range(N):
        S[n, n * BR:(n + 1) * BR] = 1.0

    Wx_aug = np.zeros((CH, KAUG), np.float32)
    Wx_aug[:, :D] = Wx
    Wx_aug[:, D] = np.asarray(b1)
    WxT = Wx_aug.T.copy()                               # (704, 256) lhsT
    wxhi = WxT.astype(ml_dtypes.bfloat16)
    wxlo = (WxT - wxhi.astype(np.float32)).astype(ml_dtypes.bfloat16)

    # sda'' = (oma/alpha)*S@dinpA; the loop's diaga@(psM + sda'') then yields
    # alpha*mem1 + oma*S@dinpA, folding the membrane decay and drive into one MM
    selw = ((oma / alpha)[None, :] * S.T).astype(np.float32)  # (256, 64) lhsT
    W2T = ((1.0 - alpha2)[:, None] * np.asarray(W2)).T.copy()  # (64, 20) lhsT
    w2hi = W2T.astype(ml_dtypes.bfloat16)
    w2lo = (W2T - w2hi.astype(np.float32)).astype(ml_dtypes.bfloat16)

    consts = dict(
        wxhi=wxhi, wxlo=wxlo,
        wsh0=Wsp[:128].T.astype(ml_dtypes.bfloat16),               # (64,128)
        wsh1=Wsp[128:].T.astype(ml_dtypes.bfloat16),
        swsmi=(oma[:, None] * (S @ Wsp)
               - VTH * np.eye(N, dtype=np.float32)).T.astype(ml_dtypes.bfloat16),
        sb0=((oma[:, None] * S[:, :128]) * beta[None, :128]).T.copy(),
        sb1=((oma[:, None] * S[:, 128:]) * beta[None, 128:]).T.copy(),
        diagb0=np.diag(beta[:128]).astype(np.float32),
        diagb1=np.diag(beta[128:]).astype(np.float32),
        diaga=np.diag(alpha).astype(np.float32),
        selw0=selw[:128].copy(), selw1=selw[128:].copy(),          # (128,64)
        w2hi=w2hi, w2lo=w2lo,
        jcc=np.ones((C, C), np.float32),
        beta0=beta[:128, None].copy(), beta1=beta[128:, None].copy(),
        omb0=omb[:128, None].copy(), omb1=omb[128:, None].copy(),
        alpha2=alpha2[:, None].copy(),
        alpha1=alpha[:, None].copy(),
    )
    blob32 = np.zeros((128, _W32), np.float32)
    for k, p, w, c0 in _SPEC32:
        blob32[:p, c0:c0 + w] = consts[k]
    blob16 = np.zeros((128, _W16), ml_dtypes.bfloat16)
    for k, p, w, c0 in _SPEC16:
        if k.startswith("wx"):
            kc = int(k[4:]); r0 = sum(KCH[:kc])
            blob16[:p, c0:c0 + w] = consts[k[:4]][r0:r0 + p, :]
        else:
            blob16[:p, c0:c0 + w] = consts[k]
    return blob32, blob16


def host_x(x_core):
    """x_core (NB,T,D) fp32 -> xt (NB, KAUG, T) bf16 with bias row."""
    nb, t, _ = x_core.shape
    xt = np.zeros((nb, KAUG, t), np.float32)
    xt[:, :D, :] = x_core.transpose(0, 2, 1)
    xt[:, D, :] = 1.0
    return xt.astype(ml_dtypes.bfloat16)


# ----------------------------------------------------------------- builder
def build(T=T_FULL, repeat=1, dinp_dve=False, z_first=False, psd_first=False):
    nc = bass.Bass()
    dp = nc.declare_dram_parameter
    xt_d = dp("xt", [NB, KAUG, T], BF16, isOutput=False)
    blob32_d = dp("blob32", [128, _W32], F32, isOutput=False)
    blob16_d = dp("blob16", [128, _W16], BF16, isOutput=False)
    m1t0_d = dp("mem1t0", [N, NB], F32, isOutput=False)
    m2t0_d = dp("mem2t0", [C, NB], F32, isOutput=False)
    out_d = dp("outT", [C, NB], F32, isOutput=True)

    TLO = 11 if T > 12 else 0   # accumulate softmax for t > 10

    with tile.TileContext(nc) as tc:
        rep_ctx = tc.For_i(0, repeat, 1) if repeat > 1 else None
        if rep_ctx is not None:
            rep_ctx.__enter__()
        with (tc.tile_pool(name="singles", bufs=1) as singles,
              tc.tile_pool(name="big", bufs=1) as big,
              tc.tile_pool(name="xst", bufs=2) as xst,
              tc.tile_pool(name="work", bufs=2) as work,
              tc.tile_pool(name="state", bufs=2) as state):
            # ---- load constants via two blob DMAs, then slice views
            blob32 = singles.tile([128, _W32], F32, tag="blob32")
            nc.sync.dma_start(out=blob32[:, :], in_=blob32_d[:, :])
            blob16 = singles.tile([128, _W16], BF16, tag="blob16")
            nc.sync.dma_start(out=blob16[:, :], in_=blob16_d[:, :])
            cons = {}
            for k, p, w, c0 in _SPEC32:
                cons[k] = blob32[0:p, c0:c0 + w]
            for k, p, w, c0 in _SPEC16:
                cons[k] = blob16[0:p, c0:c0 + w]
            cons["wxhi"] = [cons[f"wxhi{kc}"] for kc in range(6)]
            cons["wxlo"] = [cons[f"wxlo{kc}"] for kc in range(6)]
            m1t0 = singles.tile([N, NB], F32, tag="m1t0")
            nc.sync.dma_start(out=m1t0[:, :], in_=m1t0_d[:, :])
            m2t0 = singles.tile([C, NB], F32, tag="m2t0")
            nc.sync.dma_start(out=m2t0[:, :], in_=m2t0_d[:, :])

            # broadcast decay rows for scans: bb_h (128,T) = beta_h, ab2 (C,T) = alpha2
            ones0 = singles.tile([128, T], F32, tag="ones0")
            nc.vector.memset(ones0[:, :], 1.0)
            bb0 = singles.tile([128, T], F32, tag="bb0")
            nc.vector.tensor_scalar(out=bb0[:, :], in0=ones0[:, :],
                                    scalar1=cons["beta0"], scalar2=None, op0=AL.mult)
            bb1 = singles.tile([128, T], F32, tag="bb1")
            nc.vector.tensor_scalar(out=bb1[:, :], in0=ones0[:, :],
                                    scalar1=cons["beta1"], scalar2=None, op0=AL.mult)
            ab2 = singles.tile([C, T], F32, tag="ab2")
            nc.vector.tensor_scalar(out=ab2[:, :], in0=ones0[:C, :],
                                    scalar1=cons["alpha2"], scalar2=None, op0=AL.mult)

            # persistent big buffers
            sda = big.tile([N, T, NB], F32, tag="sda")          # (64, t, b)
            spkh = big.tile([N, (T + 1) * NB], BF16, tag="spkh")  # slot t+1 = spk_t
            spkh3 = spkh[:, :].rearrange("p (t b) -> p t b", b=NB)
            m2h = big.tile([C, NB * T], F32, tag="m2h")          # (20, b-major t)
            m2h3 = m2h[:, :].rearrange("p (b t) -> p b t", t=T)

            # ===== P1 in time chunks, chunk c+1 pumped into loop chunk c =====
            TC = 125
            NCHUNK = (T + TC - 1) // TC
            ps1 = tc.tile_pool(name="ps1", bufs=1, space="PSUM")
            psum1 = ps1.__enter__()
            dap = tc.tile_pool(name="dap", bufs=2)
            dap_pool = dap.__enter__()
            da_prev = {}

            def p1_gen(c, b):
                lo = c * TC
                hi = min(T, lo + TC)
                wdt = hi - lo
                xk = [xst.tile([KCH[kc], wdt], BF16, tag=f"xk{kc}",
                               name=f"xk{kc}c{c}b{b}") for kc in range(6)]
                for kc in range(6):
                    r0 = sum(KCH[:kc])
                    nc.sync.dma_start(out=xk[kc][:, :],
                                      in_=xt_d[b, r0:r0 + KCH[kc], lo:hi])
                yield
                psA = [psum1.tile([128, wdt], F32, tag=f"psA{h}",
                                  name=f"psA{h}c{c}b{b}") for h in range(2)]
                for h in range(2):
                    for kc in range(6):
                        nc.tensor.matmul(
                            psA[h][:, :],
                            cons[f"wxhi{kc}"][:, h * 128:(h + 1) * 128],
                            xk[kc][:, :], start=(kc == 0), stop=(kc == 5))
                        yield
                for h in range(2):
                    ad = work.tile([128, wdt], F32, tag=f"ad{h}", name=f"adh{h}c{c}b{b}")
                    nc.scalar.activation(out=ad[:, :], in_=psA[h][:, :],
                                         func=AF.Copy, scale=cons[f"omb{h}"])
                    init = 0.0 if c == 0 else da_prev[(b, h)][:, -1:]
                    danew = dap_pool.tile([128, wdt], F32, tag=f"da{b}_{h}",
                                          name=f"da{b}_{h}c{c}")
                    nc.vector.tensor_tensor_scan(
                        out=danew[:, :], data0=(bb0 if h == 0 else bb1)[:, lo:hi],
                        data1=ad[:, :], initial=init, op0=AL.mult, op1=AL.add)
                    da_prev[(b, h)] = danew
                    yield
                psS = psum1.tile([N, wdt], F32, tag="psS", name=f"psSc{c}b{b}")
                nc.tensor.matmul(psS[:, :], cons["selw0"], da_prev[(b, 0)][:, :],
                                 start=True, stop=False)
                nc.tensor.matmul(psS[:, :], cons["selw1"], da_prev[(b, 1)][:, :],
                                 start=False, stop=True)
                nc.scalar.copy(out=sda[:, lo:hi, b], in_=psS[:, :])
                yield

            from collections import deque
            pend = deque()

            def pump(k):
                n = 0
                while pend and n < k:
                    try:
                        next(pend[0])
                        n += 1
                    except StopIteration:
                        pend.popleft()

            # chunk 0 fully in the prologue
            for b in range(NB):
                pend.append(p1_gen(0, b))
            pump(10 ** 9)
            # ================= P4: recurrent loop =================
            # z_t = mem1_{t-1} + sda''_t carries the membrane state; the
            # diaga@z MM contributes alpha*mem1 + oma*S@dinpA in one shot.
            ps2 = tc.tile_pool(name="ps2", bufs=2, space="PSUM")
            psum = ps2.__enter__()
            nc.vector.memset(spkh3[:, 0, :], 0.0)
            dinp_prev = state.tile([128, 2 * NB], F32, tag="dinp")
            nc.vector.memset(dinp_prev[:, :], 0.0)
            z_prev = state.tile([N, NB], F32, tag="z")
            nc.vector.tensor_add(z_prev[:, :], m1t0[:, :], sda[:, 0, :])
            for t in range(T):
                c_next = t // TC + 1
                if t % TC == 0 and c_next < NCHUNK:
                    for b in range(NB):
                        pend.append(p1_gen(c_next, b))
                if t % TC == TC - 1:
                    pump(10 ** 9)      # chunk must be fully emitted before first read
                else:
                    pump(1)
                spk_prev = spkh3[:, t, :]
                psM = psum.tile([N, NB], F32, tag="psM")
                psD = psum.tile([128, 2 * NB], F32, tag="psD")

                def emit_psm():
                    nc.tensor.matmul(psM, cons["swsmi"], spk_prev, start=True, stop=False)
                    nc.tensor.matmul(psM, cons["sb0"], dinp_prev[:, 0:NB], start=False, stop=False)
                    nc.tensor.matmul(psM, cons["sb1"], dinp_prev[:, NB:2 * NB], start=False, stop=False)
                    nc.tensor.matmul(psM, cons["diaga"], z_prev[:, :], start=False, stop=True)

                def emit_psd():
                    nc.tensor.matmul(psD[:, 0:NB], cons["diagb0"], dinp_prev[:, 0:NB], start=True, stop=False)
                    nc.tensor.matmul(psD[:, 0:NB], cons["wsh0"], spk_prev, start=False, stop=False)
                    nc.tensor.matmul(psD[:, NB:2 * NB], cons["diagb1"], dinp_prev[:, NB:2 * NB], start=False, stop=False)
                    nc.tensor.matmul(psD[:, NB:2 * NB], cons["wsh1"], spk_prev, start=False, stop=True)

                if psd_first:
                    emit_psd(); emit_psm()
                else:
                    emit_psm()
                    pump(1)
                    emit_psd()
                pump(1)
                # spike + state evacuation
                def emit_spk():
                    nc.vector.tensor_scalar(out=spkh3[:, t + 1, :], in0=psM,
                                            scalar1=VTH, scalar2=None, op0=AL.is_gt)

                def emit_z():
                    z_new = state.tile([N, NB], F32, tag="z")
                    nc.vector.tensor_add(z_new[:, :], psM, sda[:, t + 1, :])
                    return z_new

                if z_first and t + 1 < T:
                    z_prev = emit_z(); emit_spk()
                else:
                    emit_spk()
                    if t + 1 < T:
                        z_prev = emit_z()
                dinp_new = state.tile([128, 2 * NB], F32, tag="dinp")
                if dinp_dve:
                    nc.vector.tensor_copy(dinp_new[:, :], psD)
                else:
                    nc.scalar.copy(out=dinp_new[:, :], in_=psD)
                dinp_prev = dinp_new

            ps2.__exit__(None, None, None)
            dap.__exit__(None, None, None)
            ps1.__exit__(None, None, None)
            # ================= P5: readout =================
            ps3 = tc.tile_pool(name="ps3", bufs=2, space="PSUM")
            psum = ps3.__enter__()
            outT = singles.tile([C, NB], F32, tag="outT")
            for b in range(NB):
                psZ = psum.tile([C, T], F32, tag="psZ")
                nc.tensor.matmul(psZ[:, :], cons["w2hi"], spkh3[:, 1:T + 1, b], start=True, stop=False)
                nc.tensor.matmul(psZ[:, :], cons["w2lo"], spkh3[:, 1:T + 1, b], start=False, stop=True)
                m2b = work.tile([C, T], F32, tag="m2b")
                nc.vector.tensor_tensor_scan(
                    out=m2b[:, :], data0=ab2[:, :], data1=psZ[:, :],
                    initial=m2t0[:, b:b + 1], op0=AL.mult, op1=AL.add)
                eb = work.tile([C, T], F32, tag="eb")
                nc.scalar.activation(out=eb[:, :], in_=m2b[:, :], func=AF.Exp)
                psS20 = psum.tile([C, T], F32, tag="psS20")
                nc.tensor.matmul(psS20[:, :], cons["jcc"], eb[:, :], start=True, stop=True)
                rb = work.tile([C, T], F32, tag="rb")
                nc.vector.reciprocal(out=rb[:, :], in_=psS20[:, :])
                scr = work.tile([C, T - TLO], F32, tag="scr")
                nc.vector.tensor_mul(scr[:, :], eb[:, TLO:], rb[:, TLO:])
                nc.vector.tensor_reduce(out=outT[:, b:b + 1], in_=scr[:, :],
                                        axis=mybir.AxisListType.X, op=AL.add)
            nc.sync.dma_start(out=out_d[:, :], in_=outT[:, :])
            ps3.__exit__(None, None, None)
        if rep_ctx is not None:
            rep_ctx.__exit__(None, None, None)
    return nc


# ----------------------------------------------------------------- entry
_CACHE = {}


def _get_nc():
    if "nc" not in _CACHE:
        _CACHE["nc"] = build(T_FULL)
    return _CACHE["nc"]


def kernel(x, W1, b1, mask, tau_m1, tau_n1, W2, b2, tau_m2, mem1_0, mem2_0):
    x = np.asarray(x, np.float32)
    blob32, blob16 = host_prep(W1, b1, mask, tau_m1, tau_n1, W2, b2, tau_m2)
    m1 = np.asarray(mem1_0, np.float32)
    m2 = np.asarray(mem2_0, np.float32)
    in_maps = []
    for c in range(NCORES):
        sl = slice(c * NB, (c + 1) * NB)
        in_maps.append(dict(
            blob32=blob32, blob16=blob16, xt=host_x(x[sl]),
            mem1t0=np.ascontiguousarray(m1[sl].T),
            mem2t0=np.ascontiguousarray(m2[sl].T)))
    nc = _get_nc()
    res = run_bass_kernel_spmd(nc, in_maps, list(range(NCORES)))
    _CACHE["last_result"] = res
    outs = [np.asarray(r["outT"]).T for r in res.results]
    return np.concatenate(outs, axis=0).astype(np.float32)


if __name__ == "__main__":
    nc = build(16)
    print("built ok; instructions:",
          sum(len(bb.instructions) for bb in nc.main_func.blocks))

